# revision 1
# baseline (speedup 1.0000x reference)
"""Causal self-attention (B=4, T=2048, C=1024, H=16) on 8 TRN2 NeuronCores.

Sharding: core c handles batch element c//2 with heads (c%2)*8 .. +8
(tensor-parallel over heads, paired per batch element).

Per-core pipeline (single SPMD Bass program, cores differ only in data),
software-pipelined per 512-token block qb:
  ph1(qb)  QKV projection for tokens [qb*512, qb*512+512): Q^T,K^T in
           [dh, t] layout, V in [t, dh] layout with an appended
           ones-column (rowsum trick), bf16 in SBUF.
  ph2(qb)  attention for query block qb: scores computed transposed
           S^T[k,q] = K-tile^T . Q^T for BOTH heads of a pair as
           back-to-back row-tiled matmuls (rows 0-63 / 64-127 of the PE
           array run concurrently), exp on ACT over a fused [128,2,512]
           tile, causal mask as 0/1 multiply on the [128,128] diagonal
           sub-tile only, exact-causal column trimming on diagonal key
           tiles, then AV matmuls with augmented V accumulate
           [y^T; rowsum] in PSUM.  Normalize into ytc (SBUF).
  ph3(qb)  partial out-projection: contracts ytc (own 512 channels)
           against own 512 rows of W_out for all 512 tokens of qb, then
           a pairwise ReduceScatter sums the two half-projections and
           leaves each core 256 of the 512 rows; DMA to out.

Host side shards inputs and reassembles the 8 output shards.
"""

import sys

import numpy as np

for _p in ("/opt/trn_rl_repo",):
    if _p not in sys.path:
        sys.path.insert(0, _p)

import ml_dtypes  # noqa: E402

import concourse.bass as bass  # noqa: E402  (unused but keeps parity)
import concourse.bacc as bacc  # noqa: E402
import concourse.mybir as mybir  # noqa: E402
import concourse.tile as tile  # noqa: E402

FP32 = mybir.dt.float32
BF16 = mybir.dt.bfloat16
Act = mybir.ActivationFunctionType

B, T, C, H, DH = 4, 2048, 1024, 16, 64
NCORES = 8
HPC = 8         # heads per core
HPT = HPC // 2  # head-pair tiles of 128 partitions
QB = 512        # token block
NQB = T // QB


def build_program(skip_bias=False):
    nc = bacc.Bacc(None, num_devices=NCORES)
    xT = nc.declare_dram_parameter("xT", [C, T], BF16, isOutput=False)
    wqkv = nc.declare_dram_parameter("wqkv", [C, 3 * 512], BF16, isOutput=False)
    wout = nc.declare_dram_parameter("wout", [512, C], BF16, isOutput=False)
    bqk = nc.declare_dram_parameter("bqk", [128, 8], FP32, isOutput=False)
    bv = nc.declare_dram_parameter("bv", [1, 512], BF16, isOutput=False)
    bhalf = nc.declare_dram_parameter("bhalf", [1, C], BF16, isOutput=False)
    tri_p = nc.declare_dram_parameter("tri", [128, 128], BF16, isOutput=False)
    onesb_p = nc.declare_dram_parameter("onesb", [1, 128], BF16, isOutput=False)
    out = nc.declare_dram_parameter("out", [T // 2, C], BF16, isOutput=True)

    groups = [[0, 1], [2, 3], [4, 5], [6, 7]]

    with tile.TileContext(nc, num_cores=NCORES) as tc:
        with (
            tc.tile_pool(name="const", bufs=1) as cpool,
            tc.tile_pool(name="dram", bufs=1, space="DRAM") as dpool,
            tc.tile_pool(name="persist", bufs=1) as ppool,
            tc.tile_pool(name="xch", bufs=2) as xpool,
            # PSUM budget (8 banks of 2KB/partition):
            #   qvp [128,512]x2 = 2 banks (phase 1 + phase 3 share)
            #   spp [128,2,512]x2 = 4 banks
            #   yap [65,1024]x1  = 2 banks
            tc.tile_pool(name="qv", bufs=2, space="PSUM") as qvp,
            tc.tile_pool(name="sp", bufs=2, space="PSUM") as spp,
            tc.tile_pool(name="yac", bufs=1, space="PSUM") as yap,
            tc.tile_pool(name="pexp", bufs=4) as pxp,
            tc.tile_pool(name="rr", bufs=4) as rrp,
            tc.tile_pool(name="ytc", bufs=2) as ytp,
            tc.tile_pool(name="ob", bufs=3) as obp,
        ):
            # ---- constants / persistent weights ----
            # Startup DMAs fan out over the SP + ACT HWDGE queues (xt/w) and
            # the Pool SWDGE queue (consts) so the first matmuls start early.
            w_sb = ppool.tile([128, 8, 3 * 512], BF16)
            tri_sb = cpool.tile([128, 128], BF16)
            nc.gpsimd.dma_start(out=tri_sb, in_=tri_p[:, :])
            onesb_sb = cpool.tile([1, 128], BF16)
            nc.gpsimd.dma_start(out=onesb_sb, in_=onesb_p[:, :])
            bqk_sb = cpool.tile([128, 8], FP32)
            nc.gpsimd.dma_start(out=bqk_sb, in_=bqk[:, :])
            bv_sb = cpool.tile([1, 512], BF16)
            nc.gpsimd.dma_start(out=bv_sb, in_=bv[:, :])
            bhalf_sb = cpool.tile([1, C], BF16)
            nc.gpsimd.dma_start(out=bhalf_sb, in_=bhalf[:, :])

            # ---- persistent activations ----
            qT_sb = ppool.tile([128, HPT, T], BF16, name="qT")
            kT_sb = ppool.tile([128, HPT, T], BF16, name="kT")
            v_sb = ppool.tile([128, HPC, T // 128, 65], BF16, name="v")
            nc.vector.memset(v_sb[:, :, :, 64], 1.0)

            # exchange buffers (dram)
            pout = [
                dpool.tile([QB, C], BF16, name=f"pout{i}") for i in range(NQB)
            ]
            rs_out = [
                dpool.tile([QB // 2, C], BF16, name=f"rsout{i}") for i in range(NQB)
            ]

            wout_sb = ppool.tile([128, HPT, C], BF16)

            def load_xt(qb, split):
                xt = xpool.tile([128, 8, QB], BF16, tag="xt")
                ts = qb * QB
                for a in range(8):
                    eng = nc.scalar if (split and a % 2) else nc.sync
                    eng.dma_start(
                        out=xt[:, a, :], in_=xT[a * 128:(a + 1) * 128, ts:ts + QB]
                    )
                return xt

            # chunk-0 x^T split over SP+ACT queues, then the qkv weights
            xt_next = load_xt(0, split=True)
            for a in range(8):
                eng = nc.scalar if a % 2 else nc.sync
                eng.dma_start(out=w_sb[:, a, :], in_=wqkv[a * 128:(a + 1) * 128, :])
            for hp in range(HPT):
                nc.gpsimd.dma_start(
                    out=wout_sb[:, hp, :], in_=wout[hp * 128:(hp + 1) * 128, :]
                )

            def phase3(qb, ytc):
                # partial out projection for qb + pairwise ReduceScatter
                for tl in range(4):
                    for co in range(2):
                        po = qvp.tile([128, 512], FP32, tag="qv")
                        for hp in range(HPT):
                            nc.tensor.matmul(
                                po,
                                lhsT=ytc[:, hp, tl * 128:(tl + 1) * 128],
                                rhs=wout_sb[:, hp, co * 512:(co + 1) * 512],
                                start=(hp == 0), stop=(skip_bias and hp == 3),
                            )
                        if not skip_bias:
                            nc.tensor.matmul(
                                po,
                                lhsT=onesb_sb[0:1, :],
                                rhs=bhalf_sb[0:1, co * 512:(co + 1) * 512],
                                start=False, stop=True,
                            )
                        ob = obp.tile([128, 512], BF16, tag="ob")
                        nc.scalar.activation(ob, po, Act.Copy)
                        nc.sync.dma_start(
                            out=pout[qb][tl * 128:(tl + 1) * 128,
                                         co * 512:(co + 1) * 512],
                            in_=ob,
                        )
                nc.gpsimd.collective_compute(
                    "ReduceScatter",
                    mybir.AluOpType.add,
                    replica_groups=groups,
                    ins=[pout[qb].opt()],
                    outs=[rs_out[qb].opt()],
                )
                # The final out-DMA for the PREVIOUS block: its RS has long
                # completed, so this never blocks the in-order SP queue the
                # way an immediate (still-pending) RS wait would.
                if qb >= 1:
                    nc.sync.dma_start(
                        out=out[(qb - 1) * 256:qb * 256, :],
                        in_=rs_out[qb - 1][:, :],
                    )

            ytc_prev = None
            for qb in range(NQB):
                ts = qb * QB
                # ============ phase 1: QKV projection for chunk qb ============
                xt = xt_next
                for kind in range(2):  # 0=q, 1=k
                    for hp in range(HPT):
                        acc = qvp.tile([128, QB], FP32, tag="qv")
                        wcol = kind * 512 + hp * 128
                        for a in range(8):
                            nc.tensor.matmul(
                                acc,
                                lhsT=w_sb[:, a, wcol:wcol + 128],
                                rhs=xt[:, a, :],
                                start=(a == 0), stop=(a == 7),
                            )
                        dst = (qT_sb if kind == 0 else kT_sb)[:, hp, ts:ts + QB]
                        if skip_bias:
                            nc.vector.tensor_copy(dst, acc)
                        else:
                            nc.vector.tensor_scalar(
                                dst, acc, 1.0,
                                bqk_sb[:, kind * 4 + hp:kind * 4 + hp + 1],
                                mybir.AluOpType.mult,
                                mybir.AluOpType.add,
                            )
                for tl in range(4):
                    accv = qvp.tile([128, QB], FP32, tag="qv")
                    for a in range(8):
                        nc.tensor.matmul(
                            accv,
                            lhsT=xt[:, a, tl * 128:(tl + 1) * 128],
                            rhs=w_sb[:, a, 1024:1536],
                            start=(a == 0), stop=(skip_bias and a == 7),
                        )
                    if not skip_bias:
                        nc.tensor.matmul(
                            accv,
                            lhsT=onesb_sb[0:1, :],
                            rhs=bv_sb[0:1, :],
                            start=False, stop=True,
                        )
                    nc.vector.tensor_copy(
                        v_sb[:, :, qb * 4 + tl, 0:64],
                        accv.rearrange("p (h d) -> p h d", d=64),
                    )

                # phase 3 of the previous block goes after this block's
                # projection so phase-1 PSUM buffers aren't gated on the
                # qb-boundary ACT/DVE backlog draining phase-3 tiles.
                if ytc_prev is not None:
                    phase3(qb - 1, ytc_prev)

                # ============ phase 2: attention for query block qb ============
                if qb + 1 < NQB:
                    xt_next = load_xt(qb + 1, split=False)
                ytc = ytp.tile([128, HPT, QB], BF16, tag="ytc")
                nkt = 4 * (qb + 1)
                for hp in range(HPT):
                    yacc = yap.tile([65, 2, QB], FP32, tag="yacc")
                    for kt in range(nkt):
                        j = kt - 4 * qb  # >= 0 on diagonal key tiles
                        qoff = max(0, j) * 128
                        sp = spp.tile([128, 2, QB], FP32, tag="sp")
                        nc.tensor.matmul(
                            sp[:, 0, qoff:QB],
                            lhsT=kT_sb[0:64, hp, kt * 128:(kt + 1) * 128],
                            rhs=qT_sb[0:64, hp, ts + qoff:ts + QB],
                        )
                        nc.tensor.matmul(
                            sp[:, 1, qoff:QB],
                            lhsT=kT_sb[64:128, hp, kt * 128:(kt + 1) * 128],
                            rhs=qT_sb[64:128, hp, ts + qoff:ts + QB],
                            tile_position=(64, 0),
                        )
                        p = pxp.tile([128, 2, QB], BF16, tag="p")
                        nc.scalar.activation(
                            p[:, :, qoff:QB], sp[:, :, qoff:QB], Act.Exp
                        )
                        if j >= 0:
                            nc.vector.tensor_mul(
                                p[:, 0, qoff:qoff + 128],
                                p[:, 0, qoff:qoff + 128], tri_sb
                            )
                            nc.vector.tensor_mul(
                                p[:, 1, qoff:qoff + 128],
                                p[:, 1, qoff:qoff + 128], tri_sb
                            )
                        nc.tensor.matmul(
                            yacc[:, 0, qoff:QB],
                            lhsT=v_sb[:, 2 * hp, kt, :],
                            rhs=p[:, 0, qoff:QB],
                            start=(kt == 0), stop=(kt == nkt - 1),
                        )
                        nc.tensor.matmul(
                            yacc[:, 1, qoff:QB],
                            lhsT=v_sb[:, 2 * hp + 1, kt, :],
                            rhs=p[:, 1, qoff:QB],
                            start=(kt == 0), stop=(kt == nkt - 1),
                        )
                    for h2 in range(2):
                        rec = rrp.tile([1, QB], FP32, tag="rec")
                        nc.vector.reciprocal(rec, yacc[64:65, h2, :])
                        rbs = rrp.tile([64, QB], FP32, tag="rbs")
                        nc.gpsimd.partition_broadcast(rbs, rec)
                        nc.vector.tensor_mul(
                            ytc[h2 * 64:h2 * 64 + 64, hp, :],
                            yacc[0:64, h2, :], rbs,
                        )

                ytc_prev = ytc
            phase3(NQB - 1, ytc_prev)
            nc.sync.dma_start(
                out=out[(NQB - 1) * 256:NQB * 256, :],
                in_=rs_out[NQB - 1][:, :],
            )
    nc.compile()
    return nc


def shard_inputs(x, W_qkv, b_qkv, W_out, b_out):
    """Build the 8 per-core input maps."""
    x = np.asarray(x, dtype=np.float32)
    W_qkv = np.asarray(W_qkv, dtype=np.float32)
    b_qkv = np.asarray(b_qkv, dtype=np.float32)
    W_out = np.asarray(W_out, dtype=np.float32)
    b_out = np.asarray(b_out, dtype=np.float32)

    tri = (np.arange(128)[None, :] >= np.arange(128)[:, None]).astype(
        ml_dtypes.bfloat16
    )
    onesb = np.ones((1, 128), dtype=ml_dtypes.bfloat16)

    in_maps = []
    for c in range(NCORES):
        b = c // 2
        hh = (c % 2) * HPC  # first head on this core
        col = hh * DH       # 512-wide column slice per kind
        xT = np.ascontiguousarray(x[b].T.astype(ml_dtypes.bfloat16))
        wq = W_qkv[:, 0 * C + col:0 * C + col + 512] * (1.0 / 8.0)
        wk = W_qkv[:, 1 * C + col:1 * C + col + 512]
        wv = W_qkv[:, 2 * C + col:2 * C + col + 512]
        wqkv_c = np.ascontiguousarray(
            np.concatenate([wq, wk, wv], axis=1).astype(ml_dtypes.bfloat16)
        )
        wout_c = np.ascontiguousarray(
            W_out[col:col + 512, :].astype(ml_dtypes.bfloat16)
        )
        bq = b_qkv[0 * C + col:0 * C + col + 512] * (1.0 / 8.0)
        bk = b_qkv[1 * C + col:1 * C + col + 512]
        bqk_c = np.stack(
            [bq[hp * 128:(hp + 1) * 128] for hp in range(4)]
            + [bk[hp * 128:(hp + 1) * 128] for hp in range(4)],
            axis=1,
        ).astype(np.float32)
        bv_c = np.ascontiguousarray(
            b_qkv[2 * C + col:2 * C + col + 512][None, :].astype(ml_dtypes.bfloat16)
        )
        bhalf = np.ascontiguousarray(
            (0.5 * b_out)[None, :].astype(ml_dtypes.bfloat16)
        )
        in_maps.append(
            {
                "xT": xT,
                "wqkv": wqkv_c,
                "wout": wout_c,
                "bqk": np.ascontiguousarray(bqk_c),
                "bv": bv_c,
                "bhalf": bhalf,
                "tri": tri,
                "onesb": onesb,
            }
        )
    return in_maps


def gather_outputs(results):
    full = np.zeros((B, T, C), dtype=np.float32)
    for c, r in enumerate(results):
        b, rk = c // 2, c % 2
        o = np.asarray(r["out"]).astype(np.float32)
        for qb in range(NQB):
            full[b, qb * 512 + rk * 256: qb * 512 + rk * 256 + 256] = o[
                qb * 256:(qb + 1) * 256
            ]
    return full


_CACHED = {}


def kernel(x, W_qkv, b_qkv, W_out, b_out):
    from concourse.bass_utils import run_bass_kernel_spmd

    zb = bool(
        np.all(np.asarray(b_qkv) == 0) and np.all(np.asarray(b_out) == 0)
    )
    key = f"nc{zb}"
    if key not in _CACHED:
        _CACHED[key] = build_program(skip_bias=zb)
    nc = _CACHED[key]
    in_maps = shard_inputs(x, W_qkv, b_qkv, W_out, b_out)
    res = run_bass_kernel_spmd(nc, in_maps, list(range(NCORES)))
    return gather_outputs(res.results)


if __name__ == "__main__":
    import reference

    inputs = reference.setup_inputs()
    expected = np.asarray(reference.reference(**inputs))
    actual = kernel(**{k: np.asarray(v) for k, v in inputs.items()})
    err = np.linalg.norm(actual - expected) / np.linalg.norm(expected)
    print("Relative error:", err)



# revision 14
# speedup vs baseline: 1.1000x; 1.1000x over previous
"""Causal self-attention (B=4, T=2048, C=1024, H=16) on 8 TRN2 NeuronCores.

Sharding: core c handles batch element c//2 with heads (c%2)*8 .. +8
(tensor-parallel over heads, paired per batch element).

Per-core pipeline (single SPMD Bass program, cores differ only in data),
software-pipelined per 512-token block qb:
  ph1(qb)  QKV projection for tokens [qb*512, qb*512+512): Q^T,K^T in
           [dh, t] layout, V in [t, dh] layout with an appended
           ones-column (rowsum trick), bf16 in SBUF.
  ph2(qb)  attention for query block qb: scores computed transposed
           S^T[k,q] = K-tile^T . Q^T for BOTH heads of a pair as
           back-to-back row-tiled matmuls (rows 0-63 / 64-127 of the PE
           array run concurrently), exp on ACT over a fused [128,2,512]
           tile, causal mask as 0/1 multiply on the [128,128] diagonal
           sub-tile only, exact-causal column trimming on diagonal key
           tiles, then AV matmuls with augmented V accumulate
           [y^T; rowsum] in PSUM.  Normalize into ytc (SBUF).
  ph3(qb)  partial out-projection: contracts ytc (own 512 channels)
           against own 512 rows of W_out for all 512 tokens of qb, then
           a pairwise ReduceScatter sums the two half-projections and
           leaves each core 256 of the 512 rows; DMA to out.

Host side shards inputs and reassembles the 8 output shards.
"""

import sys

import numpy as np

for _p in ("/opt/trn_rl_repo",):
    if _p not in sys.path:
        sys.path.insert(0, _p)

import ml_dtypes  # noqa: E402

import concourse.bass as bass  # noqa: E402  (unused but keeps parity)
import concourse.bacc as bacc  # noqa: E402
import concourse.mybir as mybir  # noqa: E402
import concourse.tile as tile  # noqa: E402

FP32 = mybir.dt.float32
BF16 = mybir.dt.bfloat16
Act = mybir.ActivationFunctionType

B, T, C, H, DH = 4, 2048, 1024, 16, 64
NCORES = 8
HPC = 8         # heads per core
HPT = HPC // 2  # head-pair tiles of 128 partitions
QB = 512        # token block
NQB = T // QB


def build_program(skip_bias=False):
    nc = bacc.Bacc(None, num_devices=NCORES)
    xT = nc.declare_dram_parameter("xT", [C, T], BF16, isOutput=False)
    wqkv = nc.declare_dram_parameter("wqkv", [C, 3 * 512], BF16, isOutput=False)
    wout = nc.declare_dram_parameter("wout", [512, C], BF16, isOutput=False)
    bqk = nc.declare_dram_parameter("bqk", [128, 8], FP32, isOutput=False)
    bv = nc.declare_dram_parameter("bv", [1, 512], BF16, isOutput=False)
    bhalf = nc.declare_dram_parameter("bhalf", [1, C], BF16, isOutput=False)
    tri_p = nc.declare_dram_parameter("tri", [128, 128], BF16, isOutput=False)
    onesb_p = nc.declare_dram_parameter("onesb", [1, 128], BF16, isOutput=False)
    out = nc.declare_dram_parameter("out", [T // 2, C], BF16, isOutput=True)

    groups = [[0, 1], [2, 3], [4, 5], [6, 7]]

    with tile.TileContext(nc, num_cores=NCORES) as tc:
        with (
            tc.tile_pool(name="const", bufs=1) as cpool,
            tc.tile_pool(name="dram", bufs=1, space="DRAM") as dpool,
            tc.tile_pool(name="persist", bufs=1) as ppool,
            tc.tile_pool(name="xch", bufs=2) as xpool,
            # PSUM budget (8 banks of 2KB/partition):
            #   qvp [128,512]x2 = 2 banks (phase 1 + phase 3 share)
            #   spp [128,2,512]x2 = 4 banks
            #   yap [65,1024]x1  = 2 banks
            tc.tile_pool(name="qv", bufs=2, space="PSUM") as qvp,
            tc.tile_pool(name="sp", bufs=2, space="PSUM") as spp,
            tc.tile_pool(name="yac", bufs=1, space="PSUM") as yap,
            tc.tile_pool(name="pexp", bufs=4) as pxp,
            tc.tile_pool(name="rr", bufs=4) as rrp,
            tc.tile_pool(name="ytc", bufs=2) as ytp,
            tc.tile_pool(name="ob", bufs=3) as obp,
        ):
            # ---- constants / persistent weights ----
            # Startup DMAs fan out over the SP + ACT HWDGE queues (xt/w) and
            # the Pool SWDGE queue (consts) so the first matmuls start early.
            w_sb = ppool.tile([128, 8, 3 * 512], BF16)
            tri_sb = cpool.tile([128, 128], BF16)
            nc.gpsimd.dma_start(out=tri_sb, in_=tri_p[:, :])
            onesb_sb = cpool.tile([1, 128], BF16)
            nc.gpsimd.dma_start(out=onesb_sb, in_=onesb_p[:, :])
            bqk_sb = cpool.tile([128, 8], FP32)
            nc.gpsimd.dma_start(out=bqk_sb, in_=bqk[:, :])
            bv_sb = cpool.tile([1, 512], BF16)
            nc.gpsimd.dma_start(out=bv_sb, in_=bv[:, :])
            bhalf_sb = cpool.tile([1, C], BF16)
            nc.gpsimd.dma_start(out=bhalf_sb, in_=bhalf[:, :])

            # ---- persistent activations ----
            qT_sb = ppool.tile([128, HPT, T], BF16, name="qT")
            kT_sb = ppool.tile([128, HPT, T], BF16, name="kT")
            v_sb = ppool.tile([128, HPC, T // 128, 65], BF16, name="v")
            nc.vector.memset(v_sb[:, :, :, 64], 1.0)

            # exchange buffers (dram)
            pout = [
                dpool.tile([QB, C], BF16, name=f"pout{i}") for i in range(NQB)
            ]
            rs_out = [
                dpool.tile([QB // 2, C], BF16, name=f"rsout{i}") for i in range(NQB)
            ]

            wout_sb = ppool.tile([128, HPT, C], BF16)

            def load_xt(qb, split):
                xt = xpool.tile([128, 8, QB], BF16, tag="xt")
                ts = qb * QB
                for a in range(8):
                    eng = nc.scalar if (split and a % 2) else nc.sync
                    eng.dma_start(
                        out=xt[:, a, :], in_=xT[a * 128:(a + 1) * 128, ts:ts + QB]
                    )
                return xt

            # chunk-0 x^T split over SP+ACT queues, then the qkv weights
            xt_next = load_xt(0, split=True)
            for a in range(8):
                eng = nc.scalar if a % 2 else nc.sync
                eng.dma_start(out=w_sb[:, a, :], in_=wqkv[a * 128:(a + 1) * 128, :])
            for hp in range(HPT):
                nc.gpsimd.dma_start(
                    out=wout_sb[:, hp, :], in_=wout[hp * 128:(hp + 1) * 128, :]
                )

            def ph1_tasks(qb, xt):
                # QKV projection for chunk qb as 12 independent PE bursts
                # (~1.7us each) to be interleaved into phase 2's exp-paced
                # key-tile loop.
                ts = qb * QB

                def qk_task(kind, hp):
                    def run():
                        acc = qvp.tile([128, QB], FP32, tag="qv")
                        wcol = kind * 512 + hp * 128
                        for a in range(8):
                            nc.tensor.matmul(
                                acc,
                                lhsT=w_sb[:, a, wcol:wcol + 128],
                                rhs=xt[:, a, :],
                                start=(a == 0), stop=(a == 7),
                            )
                        dst = (qT_sb if kind == 0 else kT_sb)[:, hp, ts:ts + QB]
                        if skip_bias:
                            nc.vector.tensor_copy(dst, acc)
                        else:
                            nc.vector.tensor_scalar(
                                dst, acc, 1.0,
                                bqk_sb[:, kind * 4 + hp:kind * 4 + hp + 1],
                                mybir.AluOpType.mult,
                                mybir.AluOpType.add,
                            )
                    return run

                def v_task(tl):
                    def run():
                        accv = qvp.tile([128, QB], FP32, tag="qv")
                        for a in range(8):
                            nc.tensor.matmul(
                                accv,
                                lhsT=xt[:, a, tl * 128:(tl + 1) * 128],
                                rhs=w_sb[:, a, 1024:1536],
                                start=(a == 0), stop=(skip_bias and a == 7),
                            )
                        if not skip_bias:
                            nc.tensor.matmul(
                                accv,
                                lhsT=onesb_sb[0:1, :],
                                rhs=bv_sb[0:1, :],
                                start=False, stop=True,
                            )
                        nc.vector.tensor_copy(
                            v_sb[:, :, qb * 4 + tl, 0:64],
                            accv.rearrange("p (h d) -> p h d", d=64),
                        )
                    return run

                return [qk_task(k, h) for k in range(2) for h in range(HPT)] \
                    + [v_task(t) for t in range(4)]

            def ph3_tasks(qb, ytc):
                # out-projection for qb as 8 PE bursts + the collective
                def po_task(tl, co):
                    def run():
                        po = qvp.tile([128, 512], FP32, tag="qv")
                        for hp in range(HPT):
                            nc.tensor.matmul(
                                po,
                                lhsT=ytc[:, hp, tl * 128:(tl + 1) * 128],
                                rhs=wout_sb[:, hp, co * 512:(co + 1) * 512],
                                start=(hp == 0), stop=(skip_bias and hp == 3),
                            )
                        if not skip_bias:
                            nc.tensor.matmul(
                                po,
                                lhsT=onesb_sb[0:1, :],
                                rhs=bhalf_sb[0:1, co * 512:(co + 1) * 512],
                                start=False, stop=True,
                            )
                        ob = obp.tile([128, 512], BF16, tag="ob")
                        nc.vector.tensor_copy(ob, po)
                        nc.sync.dma_start(
                            out=pout[qb][tl * 128:(tl + 1) * 128,
                                         co * 512:(co + 1) * 512],
                            in_=ob,
                        )
                    return run

                def coll():
                    nc.gpsimd.collective_compute(
                        "ReduceScatter",
                        mybir.AluOpType.add,
                        replica_groups=groups,
                        ins=[pout[qb].opt()],
                        outs=[rs_out[qb].opt()],
                    )
                    # out-DMA for the PREVIOUS block: its RS has long
                    # completed, so this never blocks the SP queue.
                    if qb >= 1:
                        nc.sync.dma_start(
                            out=out[(qb - 1) * 256:qb * 256, :],
                            in_=rs_out[qb - 1][:, :],
                        )

                return [po_task(tl, co) for tl in range(4) for co in range(2)] \
                    + [coll]

            def phase3(qb, ytc):
                for t in ph3_tasks(qb, ytc):
                    t()

            ytc_prev = None
            # prologue: QKV projection for chunk 0 runs undiluted
            for t in ph1_tasks(0, xt_next):
                t()
            for qb in range(NQB):
                ts = qb * QB
                # pending PE bursts to dilute into this block's phase 2:
                # next chunk's QKV projection + previous block's out-proj.
                pending = []
                if qb + 1 < NQB:
                    xt_next = load_xt(qb + 1, split=False)
                    pending += ph1_tasks(qb + 1, xt_next)
                if ytc_prev is not None:
                    pending += ph3_tasks(qb - 1, ytc_prev)

                # ============ phase 2: attention for query block qb ============
                ytc = ytp.tile([128, HPT, QB], BF16, tag="ytc")
                ytcr = ytp.tile([64, 2, HPT, QB], BF16, tag="ytcr")
                dsb = rrp.tile([128, 2, QB], FP32, tag="dsb")
                nc.gpsimd.memset(dsb, 1.0)
                nkt = 4 * (qb + 1)
                # fire pending burst i after the scores of flat-iteration
                # floor((i+1) * iters / (n+1)) — spread evenly, starting a
                # little in so the chunk-(qb+1) x DMAs have landed.
                total_iters = HPT * nkt
                fire = {}
                for i, t in enumerate(pending):
                    it = (i + 1) * total_iters // (len(pending) + 1)
                    fire.setdefault(it, []).append(t)
                for hp in range(HPT):
                    yacc = yap.tile([65, 2, QB], FP32, tag="yacc")
                    for kt in range(nkt):
                        j = kt - 4 * qb  # >= 0 on diagonal key tiles
                        qoff = max(0, j) * 128
                        sp = spp.tile([128, 2, QB], FP32, tag="sp")
                        nc.tensor.matmul(
                            sp[:, 0, qoff:QB],
                            lhsT=kT_sb[0:64, hp, kt * 128:(kt + 1) * 128],
                            rhs=qT_sb[0:64, hp, ts + qoff:ts + QB],
                        )
                        nc.tensor.matmul(
                            sp[:, 1, qoff:QB],
                            lhsT=kT_sb[64:128, hp, kt * 128:(kt + 1) * 128],
                            rhs=qT_sb[64:128, hp, ts + qoff:ts + QB],
                            tile_position=(64, 0),
                        )
                        for t in fire.get(hp * nkt + kt, []):
                            t()
                        p = pxp.tile([128, 2, QB], BF16, tag="p")
                        nc.scalar.activation(
                            p[:, :, qoff:QB], sp[:, :, qoff:QB], Act.Exp
                        )
                        if j >= 0:
                            nc.vector.tensor_mul(
                                p[:, 0, qoff:qoff + 128],
                                p[:, 0, qoff:qoff + 128], tri_sb
                            )
                            nc.vector.tensor_mul(
                                p[:, 1, qoff:qoff + 128],
                                p[:, 1, qoff:qoff + 128], tri_sb
                            )
                        nc.tensor.matmul(
                            yacc[:, 0, qoff:QB],
                            lhsT=v_sb[:, 2 * hp, kt, :],
                            rhs=p[:, 0, qoff:QB],
                            start=(kt == 0), stop=(kt == nkt - 1),
                        )
                        nc.tensor.matmul(
                            yacc[:, 1, qoff:QB],
                            lhsT=v_sb[:, 2 * hp + 1, kt, :],
                            rhs=p[:, 1, qoff:QB],
                            start=(kt == 0), stop=(kt == nkt - 1),
                        )
                    # evacuate unnormalized y (bf16) + the rowsum rows; the
                    # reciprocal is batched over all 8 heads below (a [1,512]
                    # DVE reciprocal costs ~4us — free-size x8 iterative
                    # divide — so one [8,512] call replaces eight).
                    for h2 in range(2):
                        nc.vector.tensor_copy(
                            ytcr[:, h2, hp, :],
                            yacc[0:64, h2, :],
                        )
                        nc.vector.tensor_copy(
                            dsb[32 * hp:32 * hp + 1, h2, :],
                            yacc[64:65, h2, :],
                        )

                rinv = rrp.tile([128, 2, QB], FP32, tag="rinv")
                nc.vector.reciprocal(rinv, dsb)
                for hp in range(HPT):
                    for h2 in range(2):
                        # partition_broadcast's ucode reads partition 0 at
                        # free offset 0 of its source: stage the rinv row
                        # into a base-0 [1,512] tile first.
                        rtmp = rrp.tile([1, QB], FP32, tag="rtmp")
                        nc.vector.tensor_copy(
                            rtmp, rinv[32 * hp:32 * hp + 1, h2, :]
                        )
                        rbs = rrp.tile([64, QB], FP32, tag="rbs")
                        nc.gpsimd.partition_broadcast(rbs, rtmp)
                        nc.vector.tensor_mul(
                            ytc[h2 * 64:h2 * 64 + 64, hp, :],
                            ytcr[:, h2, hp, :],
                            rbs,
                        )

                ytc_prev = ytc
            phase3(NQB - 1, ytc_prev)
            nc.sync.dma_start(
                out=out[(NQB - 1) * 256:NQB * 256, :],
                in_=rs_out[NQB - 1][:, :],
            )
    nc.compile()
    return nc


def shard_inputs(x, W_qkv, b_qkv, W_out, b_out):
    """Build the 8 per-core input maps."""
    x = np.asarray(x, dtype=np.float32)
    W_qkv = np.asarray(W_qkv, dtype=np.float32)
    b_qkv = np.asarray(b_qkv, dtype=np.float32)
    W_out = np.asarray(W_out, dtype=np.float32)
    b_out = np.asarray(b_out, dtype=np.float32)

    tri = (np.arange(128)[None, :] >= np.arange(128)[:, None]).astype(
        ml_dtypes.bfloat16
    )
    onesb = np.ones((1, 128), dtype=ml_dtypes.bfloat16)

    in_maps = []
    for c in range(NCORES):
        b = c // 2
        hh = (c % 2) * HPC  # first head on this core
        col = hh * DH       # 512-wide column slice per kind
        xT = np.ascontiguousarray(x[b].T.astype(ml_dtypes.bfloat16))
        wq = W_qkv[:, 0 * C + col:0 * C + col + 512] * (1.0 / 8.0)
        wk = W_qkv[:, 1 * C + col:1 * C + col + 512]
        wv = W_qkv[:, 2 * C + col:2 * C + col + 512]
        wqkv_c = np.ascontiguousarray(
            np.concatenate([wq, wk, wv], axis=1).astype(ml_dtypes.bfloat16)
        )
        wout_c = np.ascontiguousarray(
            W_out[col:col + 512, :].astype(ml_dtypes.bfloat16)
        )
        bq = b_qkv[0 * C + col:0 * C + col + 512] * (1.0 / 8.0)
        bk = b_qkv[1 * C + col:1 * C + col + 512]
        bqk_c = np.stack(
            [bq[hp * 128:(hp + 1) * 128] for hp in range(4)]
            + [bk[hp * 128:(hp + 1) * 128] for hp in range(4)],
            axis=1,
        ).astype(np.float32)
        bv_c = np.ascontiguousarray(
            b_qkv[2 * C + col:2 * C + col + 512][None, :].astype(ml_dtypes.bfloat16)
        )
        bhalf = np.ascontiguousarray(
            (0.5 * b_out)[None, :].astype(ml_dtypes.bfloat16)
        )
        in_maps.append(
            {
                "xT": xT,
                "wqkv": wqkv_c,
                "wout": wout_c,
                "bqk": np.ascontiguousarray(bqk_c),
                "bv": bv_c,
                "bhalf": bhalf,
                "tri": tri,
                "onesb": onesb,
            }
        )
    return in_maps


def gather_outputs(results):
    full = np.zeros((B, T, C), dtype=np.float32)
    for c, r in enumerate(results):
        b, rk = c // 2, c % 2
        o = np.asarray(r["out"]).astype(np.float32)
        for qb in range(NQB):
            full[b, qb * 512 + rk * 256: qb * 512 + rk * 256 + 256] = o[
                qb * 256:(qb + 1) * 256
            ]
    return full


_CACHED = {}


def kernel(x, W_qkv, b_qkv, W_out, b_out):
    from concourse.bass_utils import run_bass_kernel_spmd

    zb = bool(
        np.all(np.asarray(b_qkv) == 0) and np.all(np.asarray(b_out) == 0)
    )
    key = f"nc{zb}"
    if key not in _CACHED:
        _CACHED[key] = build_program(skip_bias=zb)
    nc = _CACHED[key]
    in_maps = shard_inputs(x, W_qkv, b_qkv, W_out, b_out)
    res = run_bass_kernel_spmd(nc, in_maps, list(range(NCORES)))
    return gather_outputs(res.results)


if __name__ == "__main__":
    import reference

    inputs = reference.setup_inputs()
    expected = np.asarray(reference.reference(**inputs))
    actual = kernel(**{k: np.asarray(v) for k, v in inputs.items()})
    err = np.linalg.norm(actual - expected) / np.linalg.norm(expected)
    print("Relative error:", err)



# revision 17
# speedup vs baseline: 1.3220x; 1.2017x over previous
"""Causal self-attention (B=4, T=2048, C=1024, H=16) on 8 TRN2 NeuronCores.

Sharding: core c handles batch element c//2 with heads (c%2)*8 .. +8
(tensor-parallel over heads, paired per batch element).

Per-core pipeline (single SPMD Bass program, cores differ only in data),
software-pipelined per 512-token block qb:
  ph1(qb)  QKV projection for tokens [qb*512, qb*512+512): Q^T,K^T in
           [dh, t] layout, V in [t, dh] layout with an appended
           ones-column (rowsum trick), bf16 in SBUF.
  ph2(qb)  attention for query block qb: scores computed transposed
           S^T[k,q] = K-tile^T . Q^T for BOTH heads of a pair as
           back-to-back row-tiled matmuls (rows 0-63 / 64-127 of the PE
           array run concurrently), exp on ACT over a fused [128,2,512]
           tile, causal mask as 0/1 multiply on the [128,128] diagonal
           sub-tile only, exact-causal column trimming on diagonal key
           tiles, then AV matmuls with augmented V accumulate
           [y^T; rowsum] in PSUM.  Normalize into ytc (SBUF).
  ph3(qb)  partial out-projection: contracts ytc (own 512 channels)
           against own 512 rows of W_out for all 512 tokens of qb, then
           a pairwise ReduceScatter sums the two half-projections and
           leaves each core 256 of the 512 rows; DMA to out.

Host side shards inputs and reassembles the 8 output shards.
"""

import sys

import numpy as np

for _p in ("/opt/trn_rl_repo",):
    if _p not in sys.path:
        sys.path.insert(0, _p)

import ml_dtypes  # noqa: E402

import concourse.bass as bass  # noqa: E402  (unused but keeps parity)
import concourse.bacc as bacc  # noqa: E402
import concourse.mybir as mybir  # noqa: E402
import concourse.tile as tile  # noqa: E402

FP32 = mybir.dt.float32
BF16 = mybir.dt.bfloat16
Act = mybir.ActivationFunctionType

B, T, C, H, DH = 4, 2048, 1024, 16, 64
NCORES = 8
HPC = 8         # heads per core
HPT = HPC // 2  # head-pair tiles of 128 partitions
QB = 512        # token block
NQB = T // QB


def build_program(skip_bias=False):
    nc = bacc.Bacc(None, num_devices=NCORES)
    xT = nc.declare_dram_parameter("xT", [C, T], BF16, isOutput=False)
    wqkv = nc.declare_dram_parameter("wqkv", [C, 3 * 512], BF16, isOutput=False)
    wout = nc.declare_dram_parameter("wout", [512, C], BF16, isOutput=False)
    bqk = nc.declare_dram_parameter("bqk", [128, 8], FP32, isOutput=False)
    bv = nc.declare_dram_parameter("bv", [1, 512], BF16, isOutput=False)
    bhalf = nc.declare_dram_parameter("bhalf", [1, C], BF16, isOutput=False)
    tri_p = nc.declare_dram_parameter("tri", [128, 128], BF16, isOutput=False)
    onesb_p = nc.declare_dram_parameter("onesb", [1, 128], BF16, isOutput=False)
    out = nc.declare_dram_parameter("out", [T // 2, C], BF16, isOutput=True)

    groups = [[0, 1], [2, 3], [4, 5], [6, 7]]

    with tile.TileContext(nc, num_cores=NCORES) as tc:
        with (
            tc.tile_pool(name="const", bufs=1) as cpool,
            tc.tile_pool(name="dram", bufs=1, space="DRAM") as dpool,
            tc.tile_pool(name="persist", bufs=1) as ppool,
            tc.tile_pool(name="xch", bufs=2) as xpool,
            # PSUM budget (8 banks of 2KB/partition):
            #   qvp [128,512]x2 = 2 banks (phase 1 + phase 3 share)
            #   spp [128,2,512]x2 = 4 banks
            #   yap [65,1024]x1  = 2 banks
            tc.tile_pool(name="qv", bufs=2, space="PSUM") as qvp,
            tc.tile_pool(name="sp", bufs=2, space="PSUM") as spp,
            tc.tile_pool(name="yac", bufs=1, space="PSUM") as yap,
            tc.tile_pool(name="pexp", bufs=4) as pxp,
            tc.tile_pool(name="rr", bufs=4) as rrp,
            tc.tile_pool(name="ytc", bufs=2) as ytp,
            tc.tile_pool(name="ob", bufs=3) as obp,
        ):
            # ---- constants / persistent weights ----
            # Startup DMAs fan out over the SP + ACT HWDGE queues (xt/w) and
            # the Pool SWDGE queue (consts) so the first matmuls start early.
            w_sb = ppool.tile([128, 8, 3 * 512], BF16)
            tri_sb = cpool.tile([128, 128], BF16)
            nc.gpsimd.dma_start(out=tri_sb, in_=tri_p[:, :])
            onesb_sb = cpool.tile([1, 128], BF16)
            nc.gpsimd.dma_start(out=onesb_sb, in_=onesb_p[:, :])
            bqk_sb = cpool.tile([128, 8], FP32)
            nc.gpsimd.dma_start(out=bqk_sb, in_=bqk[:, :])
            bv_sb = cpool.tile([1, 512], BF16)
            nc.gpsimd.dma_start(out=bv_sb, in_=bv[:, :])
            bhalf_sb = cpool.tile([1, C], BF16)
            nc.gpsimd.dma_start(out=bhalf_sb, in_=bhalf[:, :])

            # ---- persistent activations ----
            qT_sb = ppool.tile([128, HPT, T], BF16, name="qT")
            kT_sb = ppool.tile([128, HPT, T], BF16, name="kT")
            v_sb = ppool.tile([128, HPC, T // 128, 65], BF16, name="v")
            nc.vector.memset(v_sb[:, :, :, 64], 1.0)

            # exchange buffers (dram)
            pout = [
                dpool.tile([QB, C], BF16, name=f"pout{i}") for i in range(NQB)
            ]
            rs_out = [
                dpool.tile([QB // 2, C], BF16, name=f"rsout{i}") for i in range(NQB)
            ]

            wout_sb = ppool.tile([128, HPT, C], BF16)

            def load_xt(qb, split):
                xt = xpool.tile([128, 8, QB], BF16, tag="xt")
                ts = qb * QB
                for a in range(8):
                    eng = nc.scalar if (split and a % 2) else nc.sync
                    eng.dma_start(
                        out=xt[:, a, :], in_=xT[a * 128:(a + 1) * 128, ts:ts + QB]
                    )
                return xt

            # chunk-0 x^T split over SP+ACT queues, then the qkv weights
            xt_next = load_xt(0, split=True)
            for a in range(8):
                eng = nc.scalar if a % 2 else nc.sync
                eng.dma_start(out=w_sb[:, a, :], in_=wqkv[a * 128:(a + 1) * 128, :])
            for hp in range(HPT):
                nc.gpsimd.dma_start(
                    out=wout_sb[:, hp, :], in_=wout[hp * 128:(hp + 1) * 128, :]
                )

            def ph1_tasks(qb, xt):
                # QKV projection for chunk qb as 12 independent PE bursts
                # (~1.7us each) to be interleaved into phase 2's exp-paced
                # key-tile loop.
                ts = qb * QB

                def qk_task(kind, hp):
                    def run():
                        acc = qvp.tile([128, QB], FP32, tag="qv")
                        wcol = kind * 512 + hp * 128
                        for a in range(8):
                            nc.tensor.matmul(
                                acc,
                                lhsT=w_sb[:, a, wcol:wcol + 128],
                                rhs=xt[:, a, :],
                                start=(a == 0), stop=(a == 7),
                            )
                        dst = (qT_sb if kind == 0 else kT_sb)[:, hp, ts:ts + QB]
                        if skip_bias:
                            nc.vector.tensor_copy(dst, acc)
                        else:
                            nc.vector.tensor_scalar(
                                dst, acc, 1.0,
                                bqk_sb[:, kind * 4 + hp:kind * 4 + hp + 1],
                                mybir.AluOpType.mult,
                                mybir.AluOpType.add,
                            )
                    return run

                def v_task(tl):
                    def run():
                        accv = qvp.tile([128, QB], FP32, tag="qv")
                        for a in range(8):
                            nc.tensor.matmul(
                                accv,
                                lhsT=xt[:, a, tl * 128:(tl + 1) * 128],
                                rhs=w_sb[:, a, 1024:1536],
                                start=(a == 0), stop=(skip_bias and a == 7),
                            )
                        if not skip_bias:
                            nc.tensor.matmul(
                                accv,
                                lhsT=onesb_sb[0:1, :],
                                rhs=bv_sb[0:1, :],
                                start=False, stop=True,
                            )
                        nc.vector.tensor_copy(
                            v_sb[:, :, qb * 4 + tl, 0:64],
                            accv.rearrange("p (h d) -> p h d", d=64),
                        )
                    return run

                return [qk_task(k, h) for k in range(2) for h in range(HPT)] \
                    + [v_task(t) for t in range(4)]

            def ph3_tasks(qb, ytc):
                # out-projection for qb as 8 PE bursts + the collective
                def po_task(tl, co):
                    def run():
                        po = qvp.tile([128, 512], FP32, tag="qv")
                        for hp in range(HPT):
                            nc.tensor.matmul(
                                po,
                                lhsT=ytc[:, hp, tl * 128:(tl + 1) * 128],
                                rhs=wout_sb[:, hp, co * 512:(co + 1) * 512],
                                start=(hp == 0), stop=(skip_bias and hp == 3),
                            )
                        if not skip_bias:
                            nc.tensor.matmul(
                                po,
                                lhsT=onesb_sb[0:1, :],
                                rhs=bhalf_sb[0:1, co * 512:(co + 1) * 512],
                                start=False, stop=True,
                            )
                        ob = obp.tile([128, 512], BF16, tag="ob")
                        nc.scalar.activation(ob, po, Act.Copy)
                        nc.sync.dma_start(
                            out=pout[qb][tl * 128:(tl + 1) * 128,
                                         co * 512:(co + 1) * 512],
                            in_=ob,
                        )
                    return run

                def coll():
                    nc.gpsimd.collective_compute(
                        "ReduceScatter",
                        mybir.AluOpType.add,
                        replica_groups=groups,
                        ins=[pout[qb].opt()],
                        outs=[rs_out[qb].opt()],
                    )
                    # out-DMA for the PREVIOUS block: its RS has long
                    # completed, so this never blocks the SP queue.
                    if qb >= 1:
                        nc.sync.dma_start(
                            out=out[(qb - 1) * 256:qb * 256, :],
                            in_=rs_out[qb - 1][:, :],
                        )

                return [po_task(tl, co) for tl in range(4) for co in range(2)] \
                    + [coll]

            def phase3(qb, ytc):
                for t in ph3_tasks(qb, ytc):
                    t()

            def norm_tasks(ytc, ytcr, dsb):
                # finish the softmax normalization for a block: batched
                # reciprocal of the gathered rowsums, then per-head
                # broadcast + multiply into ytc.  Emitted as small tasks
                # interleaved into the NEXT block's phase 2 so these DVE
                # ops don't clog the in-order DVE queue ahead of the
                # causal-mask multiplies phase 2 needs.
                rinv = rrp.tile([128, 2, QB], BF16, tag="rinv")
                tasks = []
                for h2 in range(2):
                    def rec(h2=h2):
                        with nc.allow_low_precision(reason="1/d in bf16"):
                            nc.vector.reciprocal(
                                rinv[:, h2, :], dsb[:, h2, :]
                            )
                    tasks.append(rec)
                for hp in range(HPT):
                    for h2 in range(2):
                        def nm(hp=hp, h2=h2):
                            # partition_broadcast's ucode reads partition 0
                            # at free offset 0: stage the rinv row into a
                            # base-0 [1,512] tile first.
                            rtmp = rrp.tile([1, QB], BF16, tag="rtmp")
                            nc.vector.tensor_copy(
                                rtmp, rinv[32 * hp:32 * hp + 1, h2, :]
                            )
                            rbs = rrp.tile([64, QB], BF16, tag="rbs")
                            nc.gpsimd.partition_broadcast(rbs, rtmp)
                            nc.vector.tensor_mul(
                                ytc[h2 * 64:h2 * 64 + 64, hp, :],
                                ytcr[:, h2, hp, :],
                                rbs,
                            )
                        tasks.append(nm)
                return tasks

            prev = None  # (ytc, ytcr, dsb) of the previous block
            # prologue: QKV projection for chunk 0 runs undiluted
            for t in ph1_tasks(0, xt_next):
                t()
            for qb in range(NQB):
                ts = qb * QB
                # tasks to dilute into this block's phase 2: the previous
                # block's normalize + out-proj, and the next chunk's QKV.
                pending = []
                if prev is not None:
                    pending += norm_tasks(*prev)
                if qb + 1 < NQB:
                    xt_next = load_xt(qb + 1, split=False)
                    pending += ph1_tasks(qb + 1, xt_next)
                if prev is not None:
                    pending += ph3_tasks(qb - 1, prev[0])

                # ============ phase 2: attention for query block qb ============
                ytc = ytp.tile([128, HPT, QB], BF16, tag="ytc")
                ytcr = ytp.tile([64, 2, HPT, QB], BF16, tag="ytcr")
                dsb = rrp.tile([128, 2, QB], FP32, tag="dsb")
                nc.gpsimd.memset(dsb, 1.0)
                nkt = 4 * (qb + 1)
                # fire pending burst i after the scores of flat-iteration
                # floor((i+1) * iters / (n+1)) — spread evenly, starting a
                # little in so the chunk-(qb+1) x DMAs have landed.
                total_iters = HPT * nkt
                fire = {}
                for i, t in enumerate(pending):
                    it = (i + 1) * total_iters // (len(pending) + 1)
                    fire.setdefault(it, []).append(t)
                for hp in range(HPT):
                    yacc = yap.tile([65, 2, QB], FP32, tag="yacc")
                    for kt in range(nkt):
                        j = kt - 4 * qb  # >= 0 on diagonal key tiles
                        qoff = max(0, j) * 128
                        sp = spp.tile([128, 2, QB], FP32, tag="sp")
                        nc.tensor.matmul(
                            sp[:, 0, qoff:QB],
                            lhsT=kT_sb[0:64, hp, kt * 128:(kt + 1) * 128],
                            rhs=qT_sb[0:64, hp, ts + qoff:ts + QB],
                        )
                        nc.tensor.matmul(
                            sp[:, 1, qoff:QB],
                            lhsT=kT_sb[64:128, hp, kt * 128:(kt + 1) * 128],
                            rhs=qT_sb[64:128, hp, ts + qoff:ts + QB],
                            tile_position=(64, 0),
                        )
                        for t in fire.get(hp * nkt + kt, []):
                            t()
                        p = pxp.tile([128, 2, QB], BF16, tag="p")
                        nc.scalar.activation(
                            p[:, :, qoff:QB], sp[:, :, qoff:QB], Act.Exp
                        )
                        if j >= 0:
                            nc.vector.tensor_mul(
                                p[:, 0, qoff:qoff + 128],
                                p[:, 0, qoff:qoff + 128], tri_sb
                            )
                            nc.vector.tensor_mul(
                                p[:, 1, qoff:qoff + 128],
                                p[:, 1, qoff:qoff + 128], tri_sb
                            )
                        nc.tensor.matmul(
                            yacc[:, 0, qoff:QB],
                            lhsT=v_sb[:, 2 * hp, kt, :],
                            rhs=p[:, 0, qoff:QB],
                            start=(kt == 0), stop=(kt == nkt - 1),
                        )
                        nc.tensor.matmul(
                            yacc[:, 1, qoff:QB],
                            lhsT=v_sb[:, 2 * hp + 1, kt, :],
                            rhs=p[:, 1, qoff:QB],
                            start=(kt == 0), stop=(kt == nkt - 1),
                        )
                    # evacuate unnormalized y (bf16) + the rowsum rows; the
                    # reciprocal is batched over all 8 heads below (a [1,512]
                    # DVE reciprocal costs ~4us — free-size x8 iterative
                    # divide — so one [8,512] call replaces eight).
                    for h2 in range(2):
                        nc.vector.tensor_copy(
                            ytcr[:, h2, hp, :],
                            yacc[0:64, h2, :],
                        )
                        nc.vector.tensor_copy(
                            dsb[32 * hp:32 * hp + 1, h2, :],
                            yacc[64:65, h2, :],
                        )

                prev = (ytc, ytcr, dsb)
            # epilogue: last block's normalize + out-projection
            for t in norm_tasks(*prev):
                t()
            phase3(NQB - 1, prev[0])
            nc.sync.dma_start(
                out=out[(NQB - 1) * 256:NQB * 256, :],
                in_=rs_out[NQB - 1][:, :],
            )
    nc.compile()
    return nc


def shard_inputs(x, W_qkv, b_qkv, W_out, b_out):
    """Build the 8 per-core input maps."""
    x = np.asarray(x, dtype=np.float32)
    W_qkv = np.asarray(W_qkv, dtype=np.float32)
    b_qkv = np.asarray(b_qkv, dtype=np.float32)
    W_out = np.asarray(W_out, dtype=np.float32)
    b_out = np.asarray(b_out, dtype=np.float32)

    tri = (np.arange(128)[None, :] >= np.arange(128)[:, None]).astype(
        ml_dtypes.bfloat16
    )
    onesb = np.ones((1, 128), dtype=ml_dtypes.bfloat16)

    in_maps = []
    for c in range(NCORES):
        b = c // 2
        hh = (c % 2) * HPC  # first head on this core
        col = hh * DH       # 512-wide column slice per kind
        xT = np.ascontiguousarray(x[b].T.astype(ml_dtypes.bfloat16))
        wq = W_qkv[:, 0 * C + col:0 * C + col + 512] * (1.0 / 8.0)
        wk = W_qkv[:, 1 * C + col:1 * C + col + 512]
        wv = W_qkv[:, 2 * C + col:2 * C + col + 512]
        wqkv_c = np.ascontiguousarray(
            np.concatenate([wq, wk, wv], axis=1).astype(ml_dtypes.bfloat16)
        )
        wout_c = np.ascontiguousarray(
            W_out[col:col + 512, :].astype(ml_dtypes.bfloat16)
        )
        bq = b_qkv[0 * C + col:0 * C + col + 512] * (1.0 / 8.0)
        bk = b_qkv[1 * C + col:1 * C + col + 512]
        bqk_c = np.stack(
            [bq[hp * 128:(hp + 1) * 128] for hp in range(4)]
            + [bk[hp * 128:(hp + 1) * 128] for hp in range(4)],
            axis=1,
        ).astype(np.float32)
        bv_c = np.ascontiguousarray(
            b_qkv[2 * C + col:2 * C + col + 512][None, :].astype(ml_dtypes.bfloat16)
        )
        bhalf = np.ascontiguousarray(
            (0.5 * b_out)[None, :].astype(ml_dtypes.bfloat16)
        )
        in_maps.append(
            {
                "xT": xT,
                "wqkv": wqkv_c,
                "wout": wout_c,
                "bqk": np.ascontiguousarray(bqk_c),
                "bv": bv_c,
                "bhalf": bhalf,
                "tri": tri,
                "onesb": onesb,
            }
        )
    return in_maps


def gather_outputs(results):
    full = np.zeros((B, T, C), dtype=np.float32)
    for c, r in enumerate(results):
        b, rk = c // 2, c % 2
        o = np.asarray(r["out"]).astype(np.float32)
        for qb in range(NQB):
            full[b, qb * 512 + rk * 256: qb * 512 + rk * 256 + 256] = o[
                qb * 256:(qb + 1) * 256
            ]
    return full


_CACHED = {}


def kernel(x, W_qkv, b_qkv, W_out, b_out):
    from concourse.bass_utils import run_bass_kernel_spmd

    zb = bool(
        np.all(np.asarray(b_qkv) == 0) and np.all(np.asarray(b_out) == 0)
    )
    key = f"nc{zb}"
    if key not in _CACHED:
        _CACHED[key] = build_program(skip_bias=zb)
    nc = _CACHED[key]
    in_maps = shard_inputs(x, W_qkv, b_qkv, W_out, b_out)
    res = run_bass_kernel_spmd(nc, in_maps, list(range(NCORES)))
    return gather_outputs(res.results)


if __name__ == "__main__":
    import reference

    inputs = reference.setup_inputs()
    expected = np.asarray(reference.reference(**inputs))
    actual = kernel(**{k: np.asarray(v) for k, v in inputs.items()})
    err = np.linalg.norm(actual - expected) / np.linalg.norm(expected)
    print("Relative error:", err)

